# revision 60
# baseline (speedup 1.0000x reference)
"""Fused multi-head attention (B=2, T=2048, D=2048, H=16) on 8 trn2 NeuronCores.

Sharding: core c handles batch b=c//4 and heads [4g, 4g+4), g=c%4 (tensor
parallel over heads x data parallel over batch). Each core computes its
4 heads' contribution to out[b] = attn(x[b]) @ Wo^T; the host sums the 4
partials per batch.

All matmul operands are bf16 (PSUM accumulation stays f32); inputs are
cast/pre-transposed/scaled on the host. Device algorithm:
  P1a  qT = (Wq_s/sqrt(dh)) @ x^T, kT = Wk_s @ x^T        [E=512, T] bf16
  P1b  v[t,e] = x @ Wv_s^T  directly (lhsT = x^T tile)     [T, E] bf16
  P2   per i-chunk (512 queries), per surviving key tile jt, per head:
         S^T[j,i] PSUM -> exp (ACT) -> bf16 P^T (diag tiles * resident mask)
         ctx^T[e,i] += v_h^T @ P^T   (PSUM)
         l[h,i]     += sel_h^T @ P^T (one [4,512] PSUM for all 4 heads)
       epilogue: 1/l via fast-approx recip, broadcast by rank-1 matmul,
       ctx^T *= bcast(1/l) on DVE
  P3   out[t,dd] = sum_e ctx^T[e,t] * WoT[e,dd]  -> DRAM (f32)
"""

import numpy as np
import ml_dtypes

import concourse.bass as bass
import concourse.mybir as mybir
import concourse.tile as tile
from concourse import bacc
from concourse.bass_utils import run_bass_kernel_spmd

F32 = mybir.dt.float32
BF16 = mybir.dt.bfloat16
EXP = mybir.ActivationFunctionType.Exp
BF = ml_dtypes.bfloat16

B, T, D, H = 2, 2048, 2048, 16
DH = D // H          # 128
E = 512              # features per core (4 heads)
HPC = 4              # heads per core
NT = T // 128        # 16 token tiles
ND = D // 128        # 16 model-dim tiles
NE = E // 128        # 4 e-tiles per core
NI = T // 512        # 4 i-chunks (moving dim)
NJ = NT              # 16 j-tiles

_NC_CACHE = {}

# per-(jt, ic) mask-block class: 0 = fully masked (skip), 1 = unmasked
# (skip the mask multiply), 2 = mixed diagonal (multiply by resident
# pattern dm[jt - 4*ic])
SKIP, NOMULT, MIXED = 0, 1, 2


def _build(cls_key):
    cls = np.asarray(cls_key, dtype=np.int64).reshape(NJ, NI)
    nc = bacc.Bacc(None, target_bir_lowering=False, debug=False)
    # xt/wq/wk are shipped partition-major ([128, ...]) so each loads with a
    # few large DMAs instead of ~100 per-tile descriptors (608ns issue each).
    # xt column layout: nch*ND*512 + dt*512 + i ; wq/wk: dt*E + m*128 + e
    xt = nc.declare_dram_parameter("xt", [128, ND * T], BF16, isOutput=False)
    wq = nc.declare_dram_parameter("wq", [128, ND * E], BF16, isOutput=False)
    wk = nc.declare_dram_parameter("wk", [128, ND * E], BF16, isOutput=False)
    wv = nc.declare_dram_parameter("wv", [D, E], BF16, isOutput=False)
    wo = nc.declare_dram_parameter("wo", [E, D], BF16, isOutput=False)
    dm = nc.declare_dram_parameter("dm", [128, 4 * 512], BF16, isOutput=False)
    sel = nc.declare_dram_parameter("sel", [128, 16], BF16, isOutput=False)
    selb = nc.declare_dram_parameter("selb", [4, 512], BF16, isOutput=False)
    out = nc.declare_dram_parameter("out", [T, D], BF16, isOutput=True)

    with tile.TileContext(nc) as tc:
        # ---- long-lived residents (stack order: ctx outlives qk/v) -----
        pool_ctx = tc.alloc_tile_pool(name="res_ctx", bufs=1)
        ctx = [pool_ctx.tile([128, T], BF16, name=f"ctx{m}") for m in range(NE)]
        pool_v = tc.alloc_tile_pool(name="res_v", bufs=1)
        v_sb = pool_v.tile([128, NT, E], BF16)
        pool_qk = tc.alloc_tile_pool(name="res_qk", bufs=1)
        qT = [pool_qk.tile([128, T], BF16, name=f"qT{m}") for m in range(NE)]
        kT = [pool_qk.tile([128, T], BF16, name=f"kT{m}") for m in range(NE)]
        pool_c = tc.alloc_tile_pool(name="res_const", bufs=1)
        dm_sb = pool_c.tile([128, 4, 512], BF16)
        sel_sb = pool_c.tile([128, 16], BF16)
        selb_sb = pool_c.tile([4, 512], BF16)

        pool_wo = tc.alloc_tile_pool(name="res_wo", bufs=1)
        wo_sb = pool_wo.tile([128, NE, D], BF16)

        # x^T resident for both P1a and P1b
        pool_x = tc.alloc_tile_pool(name="res_x", bufs=1)
        xt_sb = pool_x.tile([128, ND * T], BF16)
        pool_wv = tc.alloc_tile_pool(name="res_wv", bufs=1)
        wv_sb = pool_wv.tile([128, ND, E], BF16)

        def xsl_ap(dt, nch):
            base = nch * ND * 512 + dt * 512
            return xt_sb[:, base:base + 512]

        def xtt_ap(dt, tt):
            base = (tt // 4) * ND * 512 + dt * 512 + (tt % 4) * 128
            return xt_sb[:, base:base + 128]

        scope_p1a = nc.named_scope("P1a_qk"); scope_p1a.__enter__()
        # ---- P1a: q/k projections --------------------------------------
        # DMA order matters: the dt=0 q/k weights + x tile go first so the
        # first matmul can start within a few us; x on the gpsimd queue in
        # parallel with weights on sync.
        p_w = tc.alloc_tile_pool(name="p1w", bufs=1)
        wq_sb = p_w.tile([128, ND * E], BF16)
        wk_sb = p_w.tile([128, ND * E], BF16)
        # weights in 4 dt-group chunks so the first matmul's slice lands fast
        for q in range(4):
            wsl = slice(q * 4 * E, (q + 1) * 4 * E)
            nc.sync.dma_start(out=wq_sb[:, wsl], in_=wq.ap()[:, wsl])
            nc.sync.dma_start(out=wk_sb[:, wsl], in_=wk.ap()[:, wsl])
        # x in nch-major chunks (first chunk split finer for fast start)
        for csl in ([slice(q * 4 * 512, (q + 1) * 4 * 512) for q in range(4)] +
                    [slice(nch * ND * 512, (nch + 1) * ND * 512)
                     for nch in range(1, NI)]):
            nc.gpsimd.dma_start(out=xt_sb[:, csl], in_=xt.ap()[:, csl])
        for dt in range(ND):
            nc.gpsimd.dma_start(out=wv_sb[:, dt, :], in_=wv.ap()[dt * 128:(dt + 1) * 128, :])
        for et in range(NE):
            nc.gpsimd.dma_start(out=wo_sb[:, et, :], in_=wo.ap()[et * 128:(et + 1) * 128, :])
        for r in range(4):
            nc.sync.dma_start(out=dm_sb[:, r, :], in_=dm.ap()[:, r * 512:(r + 1) * 512])
        nc.sync.dma_start(out=sel_sb, in_=sel.ap())
        nc.sync.dma_start(out=selb_sb, in_=selb.ap())
        p_ps1 = tc.alloc_tile_pool(name="p1ps", bufs=8, space="PSUM")
        for nch in range(NI):
            for g in range(2):      # split m into 2 groups of 2 -> 4 banks each
                ps = {}
                for m in (2 * g, 2 * g + 1):
                    ps[("q", m)] = p_ps1.tile([128, 512], F32, name="ps_q", bufs=4)
                    ps[("k", m)] = p_ps1.tile([128, 512], F32, name="ps_k", bufs=4)
                for dt in range(ND):
                    xsl = xsl_ap(dt, nch)
                    st, sp = dt == 0, dt == ND - 1
                    for m in (2 * g, 2 * g + 1):
                        wqs = slice(dt * E + m * 128, dt * E + (m + 1) * 128)
                        nc.tensor.matmul(ps[("q", m)], wq_sb[:, wqs],
                                         xsl, start=st, stop=sp)
                        nc.tensor.matmul(ps[("k", m)], wk_sb[:, wqs],
                                         xsl, start=st, stop=sp)
                for m in (2 * g, 2 * g + 1):
                    nc.scalar.copy(qT[m][:, nch * 512:(nch + 1) * 512], ps[("q", m)])
                    nc.vector.tensor_copy(kT[m][:, nch * 512:(nch + 1) * 512], ps[("k", m)])
        p_ps1.release()
        p_w.release()
        scope_p1a.__exit__(None, None, None)
        scope_p1b = nc.named_scope("P1b_v"); scope_p1b.__enter__()

        # ---- P1b: v directly in [t, e] layout --------------------------
        # only tt 0..7 standalone (needed by attention chunks 0-1); the rest
        # are emitted as interleaved units inside the fused phase
        p_ps2 = tc.alloc_tile_pool(name="p1bps", bufs=3, space="PSUM")
        for tt in range(8):
            psv = p_ps2.tile([128, 512], F32, name="ps_v")
            for dt in range(ND):
                nc.tensor.matmul(psv, xtt_ap(dt, tt),
                                 wv_sb[:, dt, :], start=(dt == 0), stop=(dt == ND - 1))
            nc.scalar.copy(v_sb[:, tt, :], psv)
        p_ps2.release()
        scope_p1b.__exit__(None, None, None)
        scope_p2 = nc.named_scope("P2_attn"); scope_p2.__enter__()

        # ---- P2+P3 fused: attention + output projection -----------------
        # P3 work for i-chunk ic becomes ready after ic's epilogue; its
        # (tt, nch) units are interleaved into the next chunk's jt loop to
        # keep the PE busy during ACT-bound stretches and epilogue stalls.
        p_pt = tc.alloc_tile_pool(name="p2pt", bufs=3)
        p_ptm = tc.alloc_tile_pool(name="p2ptm", bufs=3)
        p_ls = tc.alloc_tile_pool(name="p2ls", bufs=2)
        p_ot = tc.alloc_tile_pool(name="p3o", bufs=6)
        ps_ctx_pool = tc.alloc_tile_pool(name="p2psc", bufs=4, space="PSUM")
        ps_l_pool = tc.alloc_tile_pool(name="p2psl", bufs=1, space="PSUM")
        ps_s_pool = tc.alloc_tile_pool(name="p2pss", bufs=2, space="PSUM")
        p_ps3 = tc.alloc_tile_pool(name="p3ps", bufs=1, space="PSUM")

        # deferred work units: ("v", tt) finishes the v projection for tile
        # tt; ("o", tt, nch) is an output-projection tile
        ready = [("v", tt) for tt in range(8, NT)]
        ncopy = [0]

        def emit_p3_unit(unit, pool, tname="ps_o"):
            if unit[0] == "v":
                tt = unit[1]
                psv = pool.tile([128, 512], F32, name=tname)
                for dt in range(ND):
                    nc.tensor.matmul(psv, xtt_ap(dt, tt),
                                     wv_sb[:, dt, :],
                                     start=(dt == 0), stop=(dt == ND - 1))
                nc.scalar.copy(v_sb[:, tt, :], psv)
                return
            _, tt, nch = unit
            tsl = slice(tt * 128, (tt + 1) * 128)
            ps_o = pool.tile([128, 512], F32, name=tname)
            for et in range(NE):
                nc.tensor.matmul(
                    ps_o, ctx[et][:, tsl],
                    wo_sb[:, et, nch * 512:(nch + 1) * 512],
                    start=(et == 0), stop=(et == NE - 1))
            ot = p_ot.tile([128, 512], BF16, name="ot")
            if ncopy[0] % 2 == 0:
                nc.scalar.copy(ot, ps_o)
            else:
                nc.vector.tensor_copy(ot, ps_o)
            ncopy[0] += 1
            eng = nc.sync if ncopy[0] % 2 == 0 else nc.scalar
            eng.dma_start(
                out=out.ap()[tsl, nch * 512:(nch + 1) * 512], in_=ot)

        for ic in range(NI):
            isl = slice(ic * 512, (ic + 1) * 512)
            surv = [jt for jt in range(NJ) if cls[jt, ic] != SKIP]
            assert surv, f"i-chunk {ic}: every key block masked"
            first, last = surv[0], surv[-1]
            cps = [ps_ctx_pool.tile([128, 512], F32, name="ps_c") for _ in range(HPC)]
            lps = ps_l_pool.tile([4, 512], F32, name="ps_l")
            for idx, jt in enumerate(surv):
                mixed = cls[jt, ic] == MIXED
                r = jt - 4 * ic
                st, sp = jt == first, jt == last
                # queries i < r*128 within this i-chunk are fully masked for a
                # diagonal block: narrow all work to columns [c0:512)
                c0 = r * 128 if mixed else 0
                for h in range(HPC):
                    ps_s = ps_s_pool.tile([128, 512], F32, name="ps_s")
                    nc.tensor.matmul(
                        ps_s[:, c0:], kT[h][:, jt * 128:(jt + 1) * 128],
                        qT[h][:, ic * 512 + c0:(ic + 1) * 512],
                        start=True, stop=True)
                    pt = p_pt.tile([128, 512], BF16, name="pt")
                    nc.scalar.activation(pt[:, c0:], ps_s[:, c0:], EXP)
                    if mixed:
                        # only the first 128 surviving columns are partially
                        # masked (tril); mask them in place
                        nc.vector.tensor_mul(pt[:, c0:c0 + 128],
                                             pt[:, c0:c0 + 128],
                                             dm_sb[:, r, c0:c0 + 128])
                    nc.tensor.matmul(
                        cps[h][:, c0:], v_sb[:, jt, h * 128:(h + 1) * 128],
                        pt[:, c0:], start=st, stop=sp)
                    nc.tensor.matmul(lps[:, c0:], sel_sb[:, 4 * h:4 * h + 4],
                                     pt[:, c0:],
                                     start=(st and h == 0), stop=(sp and h == HPC - 1))
                if idx >= 1 and ready:
                    emit_p3_unit(ready.pop(0), pool=p_ps3)
            lsb = p_ls.tile([4, 512], F32, name="lsb")
            nc.scalar.copy(lsb, lps)
            rsb = p_ls.tile([4, 512], F32, name="rsb")
            nc.vector.reciprocal_approx_fast(out=rsb, in_=lsb)
            rrb = p_ls.tile([4, 512], BF16, name="rrb")
            nc.vector.tensor_copy(rrb, rsb)
            # bridge the reciprocal-chain wait with independent P3 work;
            # the S-score banks are idle between the jt loop and the bcast
            # matmuls, so borrow them to avoid serializing on one bank
            for bp, bn in ((p_ps3, "ps_o"), (ps_s_pool, "ps_s"),
                           (ps_s_pool, "ps_s")):
                if ready:
                    emit_p3_unit(ready.pop(0), pool=bp, tname=bn)
            for h in range(HPC):
                ps_b = ps_s_pool.tile([128, 512], F32, name="ps_s")
                nc.tensor.matmul(ps_b, selb_sb[:, h * 128:(h + 1) * 128], rrb,
                                 start=True, stop=True)
                bsb = p_ls.tile([128, 512], BF16, name="bsb")
                nc.vector.tensor_copy(bsb, ps_b)
                nc.vector.tensor_mul(ctx[h][:, isl], cps[h], bsb)
            ready += [("o", 4 * ic + t, nch) for t in range(4) for nch in range(NI)]
        # trailing P3 units: all attention PSUM is free now, rotate across
        # pools for full-rate pipelining
        trail_pools = [(p_ps3, "ps_o"), (ps_s_pool, "ps_s"), (ps_s_pool, "ps_s"),
                       (ps_ctx_pool, "ps_c"), (ps_ctx_pool, "ps_c")]
        k = 0
        while ready:
            pool, tname = trail_pools[k % len(trail_pools)]
            emit_p3_unit(ready.pop(0), pool=pool, tname=tname)
            k += 1
        for p in (p_ps3, ps_s_pool, ps_l_pool, ps_ctx_pool,
                  p_ot, p_ls, p_ptm, p_pt):
            p.release()
        pool_wv.release()
        pool_x.release()
        pool_wo.release()
        pool_c.release()
        pool_qk.release()
        pool_v.release()
        pool_ctx.release()
        scope_p2.__exit__(None, None, None)

    nc.compile()
    return nc


def _get_nc(cls_key):
    if cls_key not in _NC_CACHE:
        _NC_CACHE[cls_key] = _build(cls_key)
    return _NC_CACHE[cls_key]


def kernel(x, Wq, Wk, Wv, Wo, attn_mask):
    x = np.asarray(x, dtype=np.float32)
    Wq = np.asarray(Wq, dtype=np.float32)
    Wk = np.asarray(Wk, dtype=np.float32)
    Wv = np.asarray(Wv, dtype=np.float32)
    Wo = np.asarray(Wo, dtype=np.float32)
    mask = np.asarray(attn_mask, dtype=np.float32).reshape(T, T)

    emT = np.ascontiguousarray(np.exp(mask).T)  # [key j, query i]
    scale = np.float32(1.0 / np.sqrt(DH))

    blocks = emT.reshape(NJ, 128, NI, 512)
    cls = np.full((NJ, NI), MIXED, dtype=np.int64)
    for jt in range(NJ):
        for ic in range(NI):
            sub = blocks[jt, :, ic, :]
            if not sub.any():
                cls[jt, ic] = SKIP
            elif np.all(sub == 1.0):
                cls[jt, ic] = NOMULT
    # mixed blocks must sit on the block diagonal with a pattern that
    # depends only on r = jt - 4*ic (true for any causal-style mask)
    dm_pat = np.zeros((128, 4, 512), dtype=np.float32)
    seen = [False] * 4
    for jt in range(NJ):
        for ic in range(NI):
            if cls[jt, ic] != MIXED:
                continue
            r = jt - 4 * ic
            assert 0 <= r < 4, f"mixed block off-diagonal: jt={jt} ic={ic}"
            sub = blocks[jt, :, ic, :]
            if seen[r]:
                assert np.array_equal(dm_pat[:, r, :], sub)
            else:
                dm_pat[:, r, :] = sub
                seen[r] = True
    cls_key = tuple(cls.flatten().tolist())

    sel_np = np.zeros((128, 16), dtype=np.float32)
    for h in range(4):
        sel_np[:, 4 * h + h] = 1.0
    selb_np = np.zeros((4, 512), dtype=np.float32)
    for h in range(4):
        selb_np[h, h * 128:(h + 1) * 128] = 1.0

    def pmaj_w(w):
        # [D, E] -> [128, ND*E] partition-major: col = dt*E + e
        return np.ascontiguousarray(
            w.reshape(ND, 128, E).transpose(1, 0, 2).reshape(128, ND * E)
        ).astype(BF)

    def pmaj_x(xb):
        # x^T [D, T] -> [128, NI*ND*512]: col = nch*ND*512 + dt*512 + i
        return np.ascontiguousarray(
            xb.T.reshape(ND, 128, NI, 512).transpose(1, 2, 0, 3)
            .reshape(128, NI * ND * 512)).astype(BF)

    xT = [pmaj_x(x[b]) for b in range(B)]
    dm_np = np.ascontiguousarray(dm_pat.reshape(128, 4 * 512)).astype(BF)
    in_maps = []
    for c in range(8):
        b, g = c // 4, c % 4
        rows = slice(E * g, E * (g + 1))
        in_maps.append({
            "xt": xT[b],
            "wq": pmaj_w((Wq[rows, :] * scale).T),
            "wk": pmaj_w(Wk[rows, :].T),
            "wv": np.ascontiguousarray(Wv[rows, :].T).astype(BF),
            "wo": np.ascontiguousarray(Wo[:, rows].T).astype(BF),
            "dm": dm_np,
            "sel": sel_np.astype(BF),
            "selb": selb_np.astype(BF),
        })

    global _LAST_IN_MAPS, _LAST_NC
    _LAST_IN_MAPS = in_maps
    nc = _get_nc(cls_key)
    _LAST_NC = nc
    res = run_bass_kernel_spmd(nc, in_maps, list(range(8)))
    outs = [np.asarray(r["out"], dtype=np.float32) for r in res.results]
    full = np.stack([
        outs[0] + outs[1] + outs[2] + outs[3],
        outs[4] + outs[5] + outs[6] + outs[7],
    ]).astype(np.float32)
    return full


# revision 61
# speedup vs baseline: 1.0393x; 1.0393x over previous
"""Fused multi-head attention (B=2, T=2048, D=2048, H=16) on 8 trn2 NeuronCores.

Sharding: core c handles batch b=c//4 and heads [4g, 4g+4), g=c%4 (tensor
parallel over heads x data parallel over batch). Each core computes its
4 heads' contribution to out[b] = attn(x[b]) @ Wo^T; the host sums the 4
partials per batch.

All matmul operands are bf16 (PSUM accumulation stays f32); inputs are
cast/pre-transposed/scaled on the host. Device algorithm:
  P1a  qT = (Wq_s/sqrt(dh)) @ x^T, kT = Wk_s @ x^T        [E=512, T] bf16
  P1b  v[t,e] = x @ Wv_s^T  directly (lhsT = x^T tile)     [T, E] bf16
  P2   per i-chunk (512 queries), per surviving key tile jt, per head:
         S^T[j,i] PSUM -> exp (ACT) -> bf16 P^T (diag tiles * resident mask)
         ctx^T[e,i] += v_h^T @ P^T   (PSUM)
         l[h,i]     += sel_h^T @ P^T (one [4,512] PSUM for all 4 heads)
       epilogue: 1/l via fast-approx recip, broadcast by rank-1 matmul,
       ctx^T *= bcast(1/l) on DVE
  P3   out[t,dd] = sum_e ctx^T[e,t] * WoT[e,dd]  -> DRAM (f32)
"""

import numpy as np
import ml_dtypes

import concourse.bass as bass
import concourse.mybir as mybir
import concourse.tile as tile
from concourse import bacc
from concourse.bass_utils import run_bass_kernel_spmd

F32 = mybir.dt.float32
BF16 = mybir.dt.bfloat16
EXP = mybir.ActivationFunctionType.Exp
BF = ml_dtypes.bfloat16

B, T, D, H = 2, 2048, 2048, 16
DH = D // H          # 128
E = 512              # features per core (4 heads)
HPC = 4              # heads per core
NT = T // 128        # 16 token tiles
ND = D // 128        # 16 model-dim tiles
NE = E // 128        # 4 e-tiles per core
NI = T // 512        # 4 i-chunks (moving dim)
NJ = NT              # 16 j-tiles

_NC_CACHE = {}

# per-(jt, ic) mask-block class: 0 = fully masked (skip), 1 = unmasked
# (skip the mask multiply), 2 = mixed diagonal (multiply by resident
# pattern dm[jt - 4*ic])
SKIP, NOMULT, MIXED = 0, 1, 2


def _build(cls_key):
    cls = np.asarray(cls_key, dtype=np.int64).reshape(NJ, NI)
    nc = bacc.Bacc(None, target_bir_lowering=False, debug=False)
    # xt/wq/wk are shipped partition-major ([128, ...]) so each loads with a
    # few large DMAs instead of ~100 per-tile descriptors (608ns issue each).
    # xt column layout: nch*ND*512 + dt*512 + i ; wq/wk: dt*E + m*128 + e
    xt = nc.declare_dram_parameter("xt", [128, ND * T], BF16, isOutput=False)
    wq = nc.declare_dram_parameter("wq", [128, ND * E], BF16, isOutput=False)
    wk = nc.declare_dram_parameter("wk", [128, ND * E], BF16, isOutput=False)
    wv = nc.declare_dram_parameter("wv", [D, E], BF16, isOutput=False)
    wo = nc.declare_dram_parameter("wo", [E, D], BF16, isOutput=False)
    dm = nc.declare_dram_parameter("dm", [128, 4 * 512], BF16, isOutput=False)
    sel = nc.declare_dram_parameter("sel", [128, 16], BF16, isOutput=False)
    selb = nc.declare_dram_parameter("selb", [4, 512], BF16, isOutput=False)
    out = nc.declare_dram_parameter("out", [T, D], BF16, isOutput=True)

    with tile.TileContext(nc) as tc:
        # ---- long-lived residents (stack order: ctx outlives qk/v) -----
        pool_ctx = tc.alloc_tile_pool(name="res_ctx", bufs=1)
        ctx = [pool_ctx.tile([128, T], BF16, name=f"ctx{m}") for m in range(NE)]
        pool_v = tc.alloc_tile_pool(name="res_v", bufs=1)
        v_sb = pool_v.tile([128, NT, E], BF16)
        pool_qk = tc.alloc_tile_pool(name="res_qk", bufs=1)
        qT = [pool_qk.tile([128, T], BF16, name=f"qT{m}") for m in range(NE)]
        kT = [pool_qk.tile([128, T], BF16, name=f"kT{m}") for m in range(NE)]
        pool_c = tc.alloc_tile_pool(name="res_const", bufs=1)
        dm_sb = pool_c.tile([128, 4, 512], BF16)
        sel_sb = pool_c.tile([128, 16], BF16)
        selb_sb = pool_c.tile([4, 512], BF16)

        pool_wo = tc.alloc_tile_pool(name="res_wo", bufs=1)
        wo_sb = pool_wo.tile([128, NE, D], BF16)

        # x^T resident for both P1a and P1b
        pool_x = tc.alloc_tile_pool(name="res_x", bufs=1)
        xt_sb = pool_x.tile([128, ND * T], BF16)
        pool_wv = tc.alloc_tile_pool(name="res_wv", bufs=1)
        wv_sb = pool_wv.tile([128, ND, E], BF16)

        def xsl_ap(dt, nch):
            base = nch * ND * 512 + dt * 512
            return xt_sb[:, base:base + 512]

        def xtt_ap(dt, tt):
            base = (tt // 4) * ND * 512 + dt * 512 + (tt % 4) * 128
            return xt_sb[:, base:base + 128]

        scope_p1a = nc.named_scope("P1a_qk"); scope_p1a.__enter__()
        # ---- P1a: q/k projections --------------------------------------
        # DMA order matters: the dt=0 q/k weights + x tile go first so the
        # first matmul can start within a few us; x on the gpsimd queue in
        # parallel with weights on sync.
        p_w = tc.alloc_tile_pool(name="p1w", bufs=1)
        wq_sb = p_w.tile([128, ND * E], BF16)
        wk_sb = p_w.tile([128, ND * E], BF16)
        nc.sync.dma_start(out=wq_sb, in_=wq.ap())
        nc.sync.dma_start(out=wk_sb, in_=wk.ap())
        # x in nch-major chunks: the first chunk's working set arrives first
        for nch in range(NI):
            csl = slice(nch * ND * 512, (nch + 1) * ND * 512)
            nc.gpsimd.dma_start(out=xt_sb[:, csl], in_=xt.ap()[:, csl])
        for dt in range(ND):
            nc.gpsimd.dma_start(out=wv_sb[:, dt, :], in_=wv.ap()[dt * 128:(dt + 1) * 128, :])
        for et in range(NE):
            nc.gpsimd.dma_start(out=wo_sb[:, et, :], in_=wo.ap()[et * 128:(et + 1) * 128, :])
        for r in range(4):
            nc.sync.dma_start(out=dm_sb[:, r, :], in_=dm.ap()[:, r * 512:(r + 1) * 512])
        nc.sync.dma_start(out=sel_sb, in_=sel.ap())
        nc.sync.dma_start(out=selb_sb, in_=selb.ap())
        p_ps1 = tc.alloc_tile_pool(name="p1ps", bufs=8, space="PSUM")
        for nch in range(NI):
            for g in range(2):      # split m into 2 groups of 2 -> 4 banks each
                ps = {}
                for m in (2 * g, 2 * g + 1):
                    ps[("q", m)] = p_ps1.tile([128, 512], F32, name="ps_q", bufs=4)
                    ps[("k", m)] = p_ps1.tile([128, 512], F32, name="ps_k", bufs=4)
                for dt in range(ND):
                    xsl = xsl_ap(dt, nch)
                    st, sp = dt == 0, dt == ND - 1
                    for m in (2 * g, 2 * g + 1):
                        wqs = slice(dt * E + m * 128, dt * E + (m + 1) * 128)
                        nc.tensor.matmul(ps[("q", m)], wq_sb[:, wqs],
                                         xsl, start=st, stop=sp)
                        nc.tensor.matmul(ps[("k", m)], wk_sb[:, wqs],
                                         xsl, start=st, stop=sp)
                for m in (2 * g, 2 * g + 1):
                    nc.scalar.copy(qT[m][:, nch * 512:(nch + 1) * 512], ps[("q", m)])
                    nc.vector.tensor_copy(kT[m][:, nch * 512:(nch + 1) * 512], ps[("k", m)])
        p_ps1.release()
        p_w.release()
        scope_p1a.__exit__(None, None, None)
        scope_p1b = nc.named_scope("P1b_v"); scope_p1b.__enter__()

        # ---- P1b: v directly in [t, e] layout --------------------------
        # only tt 0..7 standalone (needed by attention chunks 0-1); the rest
        # are emitted as interleaved units inside the fused phase
        p_ps2 = tc.alloc_tile_pool(name="p1bps", bufs=3, space="PSUM")
        for tt in range(8):
            psv = p_ps2.tile([128, 512], F32, name="ps_v")
            for dt in range(ND):
                nc.tensor.matmul(psv, xtt_ap(dt, tt),
                                 wv_sb[:, dt, :], start=(dt == 0), stop=(dt == ND - 1))
            nc.scalar.copy(v_sb[:, tt, :], psv)
        p_ps2.release()
        scope_p1b.__exit__(None, None, None)
        scope_p2 = nc.named_scope("P2_attn"); scope_p2.__enter__()

        # ---- P2+P3 fused: attention + output projection -----------------
        # P3 work for i-chunk ic becomes ready after ic's epilogue; its
        # (tt, nch) units are interleaved into the next chunk's jt loop to
        # keep the PE busy during ACT-bound stretches and epilogue stalls.
        p_pt = tc.alloc_tile_pool(name="p2pt", bufs=3)
        p_ptm = tc.alloc_tile_pool(name="p2ptm", bufs=3)
        p_ls = tc.alloc_tile_pool(name="p2ls", bufs=2)
        p_ot = tc.alloc_tile_pool(name="p3o", bufs=6)
        ps_ctx_pool = tc.alloc_tile_pool(name="p2psc", bufs=4, space="PSUM")
        ps_l_pool = tc.alloc_tile_pool(name="p2psl", bufs=1, space="PSUM")
        ps_s_pool = tc.alloc_tile_pool(name="p2pss", bufs=2, space="PSUM")
        p_ps3 = tc.alloc_tile_pool(name="p3ps", bufs=1, space="PSUM")

        # deferred work units: ("v", tt) finishes the v projection for tile
        # tt; ("o", tt, nch) is an output-projection tile
        ready = [("v", tt) for tt in range(8, NT)]
        ncopy = [0]

        def emit_p3_unit(unit, pool, tname="ps_o"):
            if unit[0] == "v":
                tt = unit[1]
                psv = pool.tile([128, 512], F32, name=tname)
                for dt in range(ND):
                    nc.tensor.matmul(psv, xtt_ap(dt, tt),
                                     wv_sb[:, dt, :],
                                     start=(dt == 0), stop=(dt == ND - 1))
                nc.scalar.copy(v_sb[:, tt, :], psv)
                return
            _, tt, nch = unit
            tsl = slice(tt * 128, (tt + 1) * 128)
            ps_o = pool.tile([128, 512], F32, name=tname)
            for et in range(NE):
                nc.tensor.matmul(
                    ps_o, ctx[et][:, tsl],
                    wo_sb[:, et, nch * 512:(nch + 1) * 512],
                    start=(et == 0), stop=(et == NE - 1))
            ot = p_ot.tile([128, 512], BF16, name="ot")
            if ncopy[0] % 2 == 0:
                nc.scalar.copy(ot, ps_o)
            else:
                nc.vector.tensor_copy(ot, ps_o)
            ncopy[0] += 1
            eng = nc.sync if ncopy[0] % 2 == 0 else nc.scalar
            eng.dma_start(
                out=out.ap()[tsl, nch * 512:(nch + 1) * 512], in_=ot)

        for ic in range(NI):
            isl = slice(ic * 512, (ic + 1) * 512)
            surv = [jt for jt in range(NJ) if cls[jt, ic] != SKIP]
            assert surv, f"i-chunk {ic}: every key block masked"
            first, last = surv[0], surv[-1]
            cps = [ps_ctx_pool.tile([128, 512], F32, name="ps_c") for _ in range(HPC)]
            lps = ps_l_pool.tile([4, 512], F32, name="ps_l")
            for idx, jt in enumerate(surv):
                mixed = cls[jt, ic] == MIXED
                r = jt - 4 * ic
                st, sp = jt == first, jt == last
                # queries i < r*128 within this i-chunk are fully masked for a
                # diagonal block: narrow all work to columns [c0:512)
                c0 = r * 128 if mixed else 0
                for h in range(HPC):
                    ps_s = ps_s_pool.tile([128, 512], F32, name="ps_s")
                    nc.tensor.matmul(
                        ps_s[:, c0:], kT[h][:, jt * 128:(jt + 1) * 128],
                        qT[h][:, ic * 512 + c0:(ic + 1) * 512],
                        start=True, stop=True)
                    pt = p_pt.tile([128, 512], BF16, name="pt")
                    nc.scalar.activation(pt[:, c0:], ps_s[:, c0:], EXP)
                    if mixed:
                        # only the first 128 surviving columns are partially
                        # masked (tril); mask them in place
                        nc.vector.tensor_mul(pt[:, c0:c0 + 128],
                                             pt[:, c0:c0 + 128],
                                             dm_sb[:, r, c0:c0 + 128])
                    nc.tensor.matmul(
                        cps[h][:, c0:], v_sb[:, jt, h * 128:(h + 1) * 128],
                        pt[:, c0:], start=st, stop=sp)
                    nc.tensor.matmul(lps[:, c0:], sel_sb[:, 4 * h:4 * h + 4],
                                     pt[:, c0:],
                                     start=(st and h == 0), stop=(sp and h == HPC - 1))
                if idx >= 1 and ready:
                    emit_p3_unit(ready.pop(0), pool=p_ps3)
            lsb = p_ls.tile([4, 512], F32, name="lsb")
            nc.scalar.copy(lsb, lps)
            rsb = p_ls.tile([4, 512], F32, name="rsb")
            nc.vector.reciprocal_approx_fast(out=rsb, in_=lsb)
            rrb = p_ls.tile([4, 512], BF16, name="rrb")
            nc.vector.tensor_copy(rrb, rsb)
            # bridge the reciprocal-chain wait with independent P3 work;
            # the S-score banks are idle between the jt loop and the bcast
            # matmuls, so borrow them to avoid serializing on one bank
            for bp, bn in ((p_ps3, "ps_o"), (ps_s_pool, "ps_s"),
                           (ps_s_pool, "ps_s")):
                if ready:
                    emit_p3_unit(ready.pop(0), pool=bp, tname=bn)
            for h in range(HPC):
                ps_b = ps_s_pool.tile([128, 512], F32, name="ps_s")
                nc.tensor.matmul(ps_b, selb_sb[:, h * 128:(h + 1) * 128], rrb,
                                 start=True, stop=True)
                bsb = p_ls.tile([128, 512], BF16, name="bsb")
                nc.vector.tensor_copy(bsb, ps_b)
                nc.vector.tensor_mul(ctx[h][:, isl], cps[h], bsb)
            ready += [("o", 4 * ic + t, nch) for t in range(4) for nch in range(NI)]
        # trailing P3 units: all attention PSUM is free now, rotate across
        # pools for full-rate pipelining
        trail_pools = [(p_ps3, "ps_o"), (ps_s_pool, "ps_s"), (ps_s_pool, "ps_s"),
                       (ps_ctx_pool, "ps_c"), (ps_ctx_pool, "ps_c")]
        k = 0
        while ready:
            pool, tname = trail_pools[k % len(trail_pools)]
            emit_p3_unit(ready.pop(0), pool=pool, tname=tname)
            k += 1
        for p in (p_ps3, ps_s_pool, ps_l_pool, ps_ctx_pool,
                  p_ot, p_ls, p_ptm, p_pt):
            p.release()
        pool_wv.release()
        pool_x.release()
        pool_wo.release()
        pool_c.release()
        pool_qk.release()
        pool_v.release()
        pool_ctx.release()
        scope_p2.__exit__(None, None, None)

    nc.compile()
    return nc


def _get_nc(cls_key):
    if cls_key not in _NC_CACHE:
        _NC_CACHE[cls_key] = _build(cls_key)
    return _NC_CACHE[cls_key]


def kernel(x, Wq, Wk, Wv, Wo, attn_mask):
    x = np.asarray(x, dtype=np.float32)
    Wq = np.asarray(Wq, dtype=np.float32)
    Wk = np.asarray(Wk, dtype=np.float32)
    Wv = np.asarray(Wv, dtype=np.float32)
    Wo = np.asarray(Wo, dtype=np.float32)
    mask = np.asarray(attn_mask, dtype=np.float32).reshape(T, T)

    emT = np.ascontiguousarray(np.exp(mask).T)  # [key j, query i]
    scale = np.float32(1.0 / np.sqrt(DH))

    blocks = emT.reshape(NJ, 128, NI, 512)
    cls = np.full((NJ, NI), MIXED, dtype=np.int64)
    for jt in range(NJ):
        for ic in range(NI):
            sub = blocks[jt, :, ic, :]
            if not sub.any():
                cls[jt, ic] = SKIP
            elif np.all(sub == 1.0):
                cls[jt, ic] = NOMULT
    # mixed blocks must sit on the block diagonal with a pattern that
    # depends only on r = jt - 4*ic (true for any causal-style mask)
    dm_pat = np.zeros((128, 4, 512), dtype=np.float32)
    seen = [False] * 4
    for jt in range(NJ):
        for ic in range(NI):
            if cls[jt, ic] != MIXED:
                continue
            r = jt - 4 * ic
            assert 0 <= r < 4, f"mixed block off-diagonal: jt={jt} ic={ic}"
            sub = blocks[jt, :, ic, :]
            if seen[r]:
                assert np.array_equal(dm_pat[:, r, :], sub)
            else:
                dm_pat[:, r, :] = sub
                seen[r] = True
    cls_key = tuple(cls.flatten().tolist())

    sel_np = np.zeros((128, 16), dtype=np.float32)
    for h in range(4):
        sel_np[:, 4 * h + h] = 1.0
    selb_np = np.zeros((4, 512), dtype=np.float32)
    for h in range(4):
        selb_np[h, h * 128:(h + 1) * 128] = 1.0

    def pmaj_w(w):
        # [D, E] -> [128, ND*E] partition-major: col = dt*E + e
        return np.ascontiguousarray(
            w.reshape(ND, 128, E).transpose(1, 0, 2).reshape(128, ND * E)
        ).astype(BF)

    def pmaj_x(xb):
        # x^T [D, T] -> [128, NI*ND*512]: col = nch*ND*512 + dt*512 + i
        return np.ascontiguousarray(
            xb.T.reshape(ND, 128, NI, 512).transpose(1, 2, 0, 3)
            .reshape(128, NI * ND * 512)).astype(BF)

    xT = [pmaj_x(x[b]) for b in range(B)]
    dm_np = np.ascontiguousarray(dm_pat.reshape(128, 4 * 512)).astype(BF)
    in_maps = []
    for c in range(8):
        b, g = c // 4, c % 4
        rows = slice(E * g, E * (g + 1))
        in_maps.append({
            "xt": xT[b],
            "wq": pmaj_w((Wq[rows, :] * scale).T),
            "wk": pmaj_w(Wk[rows, :].T),
            "wv": np.ascontiguousarray(Wv[rows, :].T).astype(BF),
            "wo": np.ascontiguousarray(Wo[:, rows].T).astype(BF),
            "dm": dm_np,
            "sel": sel_np.astype(BF),
            "selb": selb_np.astype(BF),
        })

    global _LAST_IN_MAPS, _LAST_NC
    _LAST_IN_MAPS = in_maps
    nc = _get_nc(cls_key)
    _LAST_NC = nc
    res = run_bass_kernel_spmd(nc, in_maps, list(range(8)))
    outs = [np.asarray(r["out"], dtype=np.float32) for r in res.results]
    full = np.stack([
        outs[0] + outs[1] + outs[2] + outs[3],
        outs[4] + outs[5] + outs[6] + outs[7],
    ]).astype(np.float32)
    return full
